# revision 19
# baseline (speedup 1.0000x reference)
"""nn_ContinualLearningNetwork — Trainium2 Bass SPMD kernel (8 NeuronCores).

Structure:
  Stage A (bass, 8 cores, batch-sharded): feature MLP 2048->512->256->256, fp32,
    transposed-activation layout (activations [dim, batch] on chip).
  Host glue (jnp, same backend/ops as the reference): memory scores + top-k.
    The top-k ordering of 100k scores has near-ties at fp32 resolution; the
    B*m_sq term amplifies m_sq rounding 2048x, so m_sq/scores must be computed
    with the reference's own op sequence to reproduce its ordering.
  Stage B (bass, 8 cores, batch-sharded): task MLP 512->2048->2048->1024, fp32.
"""

import numpy as np

import concourse.bacc as bacc
import concourse.mybir as mybir
import concourse.tile as tile
from concourse.bass_utils import run_bass_kernel_spmd

P = 128
NCORES = 8
B = 2048
IN_DIM = 2048
FEAT_DIM = 256
HID = 2048
OUT_DIM = 1024
NB = B // NCORES  # 256 batch rows per core

F32 = mybir.dt.float32
RELU = mybir.ActivationFunctionType.Relu
COPY = mybir.ActivationFunctionType.Identity


def _mlp_kernel(nc, tc, pool, psum, rhs_tiles, w_all, b_all, k_chunks, m_chunks,
                relu, out_prefix, mm_dt=None):
    """Emit one dense layer: out[m][128, NB] = act(W.T @ rhs + b).

    rhs_tiles: list of k_chunks SBUF tiles [P, NB] (transposed activations).
    w_all: SBUF tile [P, k_chunks, M] (natural [K, M] weight layout).
    b_all: SBUF tile [P, m_chunks] (bias, partition-major per m-chunk).
    mm_dt: bitcast matmul operands to this dtype (float32r runs 4x faster
      than float32 at ~1.5e-4 precision; fine for the task MLP, not for the
      feature MLP whose output feeds the top-k score ordering).
    Returns list of m_chunks SBUF tiles [P, NB].
    """
    outs = []
    for m in range(m_chunks):
        acc = psum.tile([P, NB], F32, tag="acc")
        for k in range(k_chunks):
            nc.tensor.matmul(
                acc[:],
                w_all[:, k, m * P:(m + 1) * P],
                rhs_tiles[k][:],
                start=(k == 0),
                stop=(k == k_chunks - 1),
            )
        o_t = pool.tile([P, NB], mm_dt or F32, tag=f"{out_prefix}{m}")
        nc.scalar.activation(o_t[:], acc[:], RELU if relu else COPY,
                             bias=b_all[:, m:m + 1])
        outs.append(o_t)
    return outs


def _build_stage_a():
    nc = bacc.Bacc(None, target_bir_lowering=False)
    xT = nc.declare_dram_parameter("xT", [IN_DIM, NB], F32, isOutput=False)
    W1 = nc.declare_dram_parameter("W1", [IN_DIM, 512], F32, isOutput=False)
    b1 = nc.declare_dram_parameter("b1", [512], F32, isOutput=False)
    W2 = nc.declare_dram_parameter("W2", [512, 256], F32, isOutput=False)
    b2 = nc.declare_dram_parameter("b2", [256], F32, isOutput=False)
    W3 = nc.declare_dram_parameter("W3", [256, FEAT_DIM], F32, isOutput=False)
    b3 = nc.declare_dram_parameter("b3", [FEAT_DIM], F32, isOutput=False)
    fT = nc.declare_dram_parameter("fT", [FEAT_DIM, NB], F32, isOutput=True)

    KC1, MC1 = IN_DIM // P, 512 // P    # 16, 4
    KC2, MC2 = 512 // P, 256 // P       # 4, 2
    KC3, MC3 = 256 // P, FEAT_DIM // P  # 2, 2

    with tile.TileContext(nc) as tc:
        with (
            tc.tile_pool(name="sbuf", bufs=1) as pool,
            tc.tile_pool(name="psum", bufs=4, space="PSUM") as psum,
        ):
            engs = [nc.gpsimd, nc.sync, nc.scalar]
            x_tiles = []
            for k in range(KC1):
                t = pool.tile([P, NB], F32, tag=f"x{k}")
                engs[k % 2].dma_start(t[:], xT[k * P:(k + 1) * P, :])
                x_tiles.append(t)
            w1_all = pool.tile([P, KC1, 512], F32, tag="w1")
            w1_r = W1.rearrange("(c p) m -> p c m", p=P)
            for k in range(KC1):
                engs[1 + k % 2].dma_start(w1_all[:, k, :], w1_r[:, k, :])
            w2_all = pool.tile([P, KC2, 256], F32, tag="w2")
            nc.gpsimd.dma_start(w2_all[:], W2.rearrange("(c p) m -> p c m", p=P))
            w3_all = pool.tile([P, KC3, FEAT_DIM], F32, tag="w3")
            nc.gpsimd.dma_start(w3_all[:], W3.rearrange("(c p) m -> p c m", p=P))
            b1_all = pool.tile([P, MC1], F32, tag="b1")
            nc.gpsimd.dma_start(b1_all[:], b1.rearrange("(mc p) -> p mc", p=P))
            b2_all = pool.tile([P, MC2], F32, tag="b2")
            nc.gpsimd.dma_start(b2_all[:], b2.rearrange("(mc p) -> p mc", p=P))
            b3_all = pool.tile([P, MC3], F32, tag="b3")
            nc.gpsimd.dma_start(b3_all[:], b3.rearrange("(mc p) -> p mc", p=P))

            h1 = _mlp_kernel(nc, tc, pool, psum, x_tiles, w1_all, b1_all,
                             KC1, MC1, True, "h1_")
            h2 = _mlp_kernel(nc, tc, pool, psum, h1, w2_all, b2_all,
                             KC2, MC2, True, "h2_")
            f_tiles = _mlp_kernel(nc, tc, pool, psum, h2, w3_all, b3_all,
                                  KC3, MC3, False, "f_")
            for m in range(MC3):
                nc.gpsimd.dma_start(fT[m * P:(m + 1) * P, :], f_tiles[m][:])

    nc.finalize()
    return nc


def _build_stage_b():
    """Task MLP, fp32r. Layer 1 keeps weights stationary (form B, small
    weights). Layers 2/3 make the ACTIVATIONS stationary and stream the big
    weight matrices as the moving operand with N=512 — this cuts LDWEIGHTS
    count 8x (LDWEIGHTS was half of all PE time in form B). The price is a
    PE-transpose of activations between layers 2 and 3, and that layer-2/3
    outputs are [batch, dim] (natural) so the final output DMAs out natural.
    """
    nc = bacc.Bacc(None, target_bir_lowering=False)
    F32R = mybir.dt.float32r
    hT = nc.declare_dram_parameter("hT", [2 * FEAT_DIM, NB], F32R, isOutput=False)
    Wt1 = nc.declare_dram_parameter("Wt1", [2 * FEAT_DIM, HID], F32R, isOutput=False)
    bt1 = nc.declare_dram_parameter("bt1", [HID], F32, isOutput=False)
    Wt2 = nc.declare_dram_parameter("Wt2", [HID, HID], F32R, isOutput=False)
    bt2 = nc.declare_dram_parameter("bt2", [HID], F32, isOutput=False)
    Wo = nc.declare_dram_parameter("Wo", [HID, OUT_DIM], F32R, isOutput=False)
    bo = nc.declare_dram_parameter("bo", [OUT_DIM], F32, isOutput=False)
    out = nc.declare_dram_parameter("out", [NB, OUT_DIM], F32, isOutput=True)
    KC1, MC1 = (2 * FEAT_DIM) // P, HID // P  # 4, 16
    KH = HID // P                             # 16
    BC = NB // P                              # 2 batch chunks
    NW2 = HID // 512                          # 4 col-chunks (L2)
    NW3 = OUT_DIM // 512                      # 2 col-chunks (L3)

    from concourse.masks import make_identity

    with tile.TileContext(nc) as tc:
        with (
            tc.tile_pool(name="sbuf", bufs=1) as pool,
            tc.tile_pool(name="stream", bufs=4) as stream,
        ):
            engs = [nc.gpsimd, nc.sync, nc.scalar]
            h_tiles = []
            for k in range(KC1):
                t = pool.tile([P, NB], F32R, tag=f"h{k}")
                nc.gpsimd.dma_start(t[:], hT[k * P:(k + 1) * P, :])
                h_tiles.append(t)
            wt1_all = pool.tile([P, KC1, HID], F32R, tag="wt1")
            wt1_r = Wt1.rearrange("(c p) m -> p c m", p=P)
            for k in range(KC1):
                engs[(k + 1) % 2].dma_start(wt1_all[:, k, :], wt1_r[:, k, :])
            bt1_all = pool.tile([P, MC1], F32, tag="bt1")
            nc.gpsimd.dma_start(bt1_all[:], bt1.rearrange("(mc p) -> p mc", p=P))
            # biases for layers 2/3 sit on the FREE axis (natural layout), so
            # replicate them across all partitions via a broadcast DMA
            bt2_rep = pool.tile([P, HID], F32, tag="bt2")
            nc.gpsimd.dma_start(
                bt2_rep[:],
                bt2.rearrange("(o m) -> o m", o=1).to_broadcast((P, HID)))
            bo_rep = pool.tile([P, OUT_DIM], F32, tag="bo")
            nc.gpsimd.dma_start(
                bo_rep[:],
                bo.rearrange("(o m) -> o m", o=1).to_broadcast((P, OUT_DIM)))

            ident_f = pool.tile([P, P], F32, tag="identf")
            make_identity(nc, ident_f[:])
            ident_r = pool.tile([P, P], F32R, tag="identr")
            nc.vector.tensor_copy(ident_r[:], ident_f[:])

            # ---- layer 1 (form B): h2T[k] = relu(Wt1.T @ h + bt1), [P, NB]
            with tc.tile_pool(name="psum_a", bufs=2, space="PSUM") as psum_a:
                h2 = _mlp_kernel(nc, tc, pool, psum_a, h_tiles, wt1_all, bt1_all,
                                 KC1, MC1, True, "h2_", mm_dt=F32R)

            # ---- layer 2 (form A): h3_nat[b][n] = relu(h2T.T @ Wt2 + bt2)
            h3_nat = [[None] * NW2 for _ in range(BC)]
            with tc.tile_pool(name="psum_b", bufs=1, space="PSUM") as psum_b:
                accs = {}
                for k in range(KH):
                    w_blk = stream.tile([P, HID], F32R, tag="wt2row")
                    engs[k % 2].dma_start(w_blk[:], Wt2[k * P:(k + 1) * P, :])
                    for b in range(BC):
                        for n in range(NW2):
                            if k == 0:
                                accs[b, n] = psum_b.tile([P, 512], F32, name=f"l2a{b}{n}",
                                                         tag=f"l2a{b}{n}")
                            nc.tensor.matmul(
                                accs[b, n][:],
                                h2[k][:, b * P:(b + 1) * P],
                                w_blk[:, n * 512:(n + 1) * 512],
                                start=(k == 0), stop=(k == KH - 1))
                for b in range(BC):
                    for n in range(NW2):
                        tmp = pool.tile([P, 512], F32, tag="l2tmp")
                        nc.vector.tensor_add(tmp[:], accs[b, n][:],
                                             bt2_rep[:, n * 512:(n + 1) * 512])
                        h3_nat[b][n] = pool.tile([P, 512], F32R, name=f"h3n{b}{n}",
                                                 tag=f"h3n{b}{n}")
                        nc.scalar.activation(h3_nat[b][n][:], tmp[:], RELU)

            # ---- transpose h3_nat -> h3T[k] [P, NB] (hid on partitions)
            h3T = [pool.tile([P, NB], F32R, name=f"h3T{k}", tag=f"h3T{k}") for k in range(KH)]
            with (
                tc.tile_pool(name="psum_t", bufs=4, space="PSUM") as psum_t,
                tc.tile_pool(name="psum_c", bufs=1, space="PSUM") as psum_c,
            ):
                for b in range(BC):
                    for kk in range(KH):
                        n, c = divmod(kk, 4)
                        ps = psum_t.tile([P, P], F32R, tag="tp")
                        nc.tensor.transpose(
                            ps[:], h3_nat[b][n][:, c * P:(c + 1) * P], ident_r[:])
                        nc.vector.tensor_copy(h3T[kk][:, b * P:(b + 1) * P], ps[:])

                # ---- layer 3 (form A): out[b][n] = h3T.T @ Wo + bo
                accs3 = {}
                for k in range(KH):
                    w_blk = stream.tile([P, OUT_DIM], F32R, tag="worow")
                    engs[k % 2].dma_start(w_blk[:], Wo[k * P:(k + 1) * P, :])
                    for b in range(BC):
                        for n in range(NW3):
                            if k == 0:
                                accs3[b, n] = psum_c.tile([P, 512], F32, name=f"l3a{b}{n}",
                                                          tag=f"l3a{b}{n}")
                            nc.tensor.matmul(
                                accs3[b, n][:],
                                h3T[k][:, b * P:(b + 1) * P],
                                w_blk[:, n * 512:(n + 1) * 512],
                                start=(k == 0), stop=(k == KH - 1))
                for b in range(BC):
                    for n in range(NW3):
                        o_t = pool.tile([P, 512], F32, tag="l3o")
                        nc.vector.tensor_add(o_t[:], accs3[b, n][:],
                                             bo_rep[:, n * 512:(n + 1) * 512])
                        nc.gpsimd.dma_start(
                            out[b * P:(b + 1) * P, n * 512:(n + 1) * 512], o_t[:])

    nc.finalize()
    return nc


_CACHE = {}


def _stage(name, builder):
    if name not in _CACHE:
        _CACHE[name] = builder()
    return _CACHE[name]


def kernel(x, W1, b1, W2, b2, W3, b3, memory, Wt1, bt1, Wt2, bt2, Wo, bo,
           _collect=None, _trace=False):
    x = np.asarray(x, dtype=np.float32)
    W1 = np.asarray(W1, dtype=np.float32); b1 = np.asarray(b1, dtype=np.float32)
    W2 = np.asarray(W2, dtype=np.float32); b2 = np.asarray(b2, dtype=np.float32)
    W3 = np.asarray(W3, dtype=np.float32); b3 = np.asarray(b3, dtype=np.float32)
    memory = np.asarray(memory, dtype=np.float32)
    Wt1 = np.asarray(Wt1, dtype=np.float32); bt1 = np.asarray(bt1, dtype=np.float32)
    Wt2 = np.asarray(Wt2, dtype=np.float32); bt2 = np.asarray(bt2, dtype=np.float32)
    Wo = np.asarray(Wo, dtype=np.float32); bo = np.asarray(bo, dtype=np.float32)

    # ---- stage A: feature MLP on 8 cores (batch-sharded) ----
    xT = np.ascontiguousarray(x.T)  # [IN_DIM, B]
    nc_a = _stage("a", _build_stage_a)
    in_maps = [
        {"xT": np.ascontiguousarray(xT[:, c * NB:(c + 1) * NB]),
         "W1": W1, "b1": b1, "W2": W2, "b2": b2, "W3": W3, "b3": b3}
        for c in range(NCORES)
    ]
    res_a = run_bass_kernel_spmd(nc_a, in_maps, list(range(NCORES)), trace=_trace)
    fT = np.concatenate([res_a.results[c]["fT"] for c in range(NCORES)], axis=1)
    f = np.ascontiguousarray(fT.T)  # [B, FEAT_DIM]

    # ---- host glue: scores + top-k, mirroring the reference's ops ----
    import jax
    import jax.numpy as jnp
    fd = jnp.asarray(f)
    md = jnp.asarray(memory)
    bsz = B
    f_sum = jnp.sum(fd, axis=0)
    f_sq = jnp.sum(fd * fd)
    m_sq = jnp.sum(md * md, axis=1)
    scores = -(bsz * m_sq - 2.0 * (md @ f_sum) + f_sq)
    _, idx = jax.lax.top_k(scores, B)
    idx = np.asarray(idx)
    mem_out = memory[idx]  # [B, FEAT_DIM]

    # ---- stage B: task MLP on 8 cores (batch-sharded) ----
    hT = np.ascontiguousarray(np.concatenate([f, mem_out], axis=1).T)  # [512, B]
    nc_b = _stage("b", _build_stage_b)
    in_maps = [
        {"hT": np.ascontiguousarray(hT[:, c * NB:(c + 1) * NB]),
         "Wt1": Wt1, "bt1": bt1, "Wt2": Wt2, "bt2": bt2, "Wo": Wo, "bo": bo}
        for c in range(NCORES)
    ]
    res_b = run_bass_kernel_spmd(nc_b, in_maps, list(range(NCORES)), trace=_trace)
    out = np.concatenate([res_b.results[c]["out"] for c in range(NCORES)], axis=0)

    if _collect is not None:
        _collect["f"] = f
        _collect["idx"] = idx
        _collect["res_a"] = res_a
        _collect["res_b"] = res_b
    return out


# revision 20
# speedup vs baseline: 1.0333x; 1.0333x over previous
"""nn_ContinualLearningNetwork — Trainium2 Bass SPMD kernel (8 NeuronCores).

Structure:
  Stage A (bass, 8 cores, batch-sharded): feature MLP 2048->512->256->256, fp32,
    transposed-activation layout (activations [dim, batch] on chip).
  Host glue (jnp, same backend/ops as the reference): memory scores + top-k.
    The top-k ordering of 100k scores has near-ties at fp32 resolution; the
    B*m_sq term amplifies m_sq rounding 2048x, so m_sq/scores must be computed
    with the reference's own op sequence to reproduce its ordering.
  Stage B (bass, 8 cores, batch-sharded): task MLP 512->2048->2048->1024, fp32.
"""

import numpy as np

import concourse.bacc as bacc
import concourse.mybir as mybir
import concourse.tile as tile
from concourse.bass_utils import run_bass_kernel_spmd

P = 128
NCORES = 8
B = 2048
IN_DIM = 2048
FEAT_DIM = 256
HID = 2048
OUT_DIM = 1024
NB = B // NCORES  # 256 batch rows per core

F32 = mybir.dt.float32
RELU = mybir.ActivationFunctionType.Relu
COPY = mybir.ActivationFunctionType.Identity


def _mlp_kernel(nc, tc, pool, psum, rhs_tiles, w_all, b_all, k_chunks, m_chunks,
                relu, out_prefix, mm_dt=None):
    """Emit one dense layer: out[m][128, NB] = act(W.T @ rhs + b).

    rhs_tiles: list of k_chunks SBUF tiles [P, NB] (transposed activations).
    w_all: SBUF tile [P, k_chunks, M] (natural [K, M] weight layout).
    b_all: SBUF tile [P, m_chunks] (bias, partition-major per m-chunk).
    mm_dt: bitcast matmul operands to this dtype (float32r runs 4x faster
      than float32 at ~1.5e-4 precision; fine for the task MLP, not for the
      feature MLP whose output feeds the top-k score ordering).
    Returns list of m_chunks SBUF tiles [P, NB].
    """
    outs = []
    for m in range(m_chunks):
        acc = psum.tile([P, NB], F32, tag="acc")
        for k in range(k_chunks):
            nc.tensor.matmul(
                acc[:],
                w_all[:, k, m * P:(m + 1) * P],
                rhs_tiles[k][:],
                start=(k == 0),
                stop=(k == k_chunks - 1),
            )
        o_t = pool.tile([P, NB], mm_dt or F32, tag=f"{out_prefix}{m}")
        nc.scalar.activation(o_t[:], acc[:], RELU if relu else COPY,
                             bias=b_all[:, m:m + 1])
        outs.append(o_t)
    return outs


def _build_stage_a():
    nc = bacc.Bacc(None, target_bir_lowering=False)
    xT = nc.declare_dram_parameter("xT", [IN_DIM, NB], F32, isOutput=False)
    W1 = nc.declare_dram_parameter("W1", [IN_DIM, 512], F32, isOutput=False)
    b1 = nc.declare_dram_parameter("b1", [512], F32, isOutput=False)
    W2 = nc.declare_dram_parameter("W2", [512, 256], F32, isOutput=False)
    b2 = nc.declare_dram_parameter("b2", [256], F32, isOutput=False)
    W3 = nc.declare_dram_parameter("W3", [256, FEAT_DIM], F32, isOutput=False)
    b3 = nc.declare_dram_parameter("b3", [FEAT_DIM], F32, isOutput=False)
    fT = nc.declare_dram_parameter("fT", [FEAT_DIM, NB], F32, isOutput=True)

    KC1, MC1 = IN_DIM // P, 512 // P    # 16, 4
    KC2, MC2 = 512 // P, 256 // P       # 4, 2
    KC3, MC3 = 256 // P, FEAT_DIM // P  # 2, 2

    with tile.TileContext(nc) as tc:
        with (
            tc.tile_pool(name="sbuf", bufs=1) as pool,
            tc.tile_pool(name="psum", bufs=4, space="PSUM") as psum,
        ):
            engs = [nc.gpsimd, nc.sync, nc.scalar]
            x_tiles = []
            for k in range(KC1):
                t = pool.tile([P, NB], F32, tag=f"x{k}")
                engs[k % 2].dma_start(t[:], xT[k * P:(k + 1) * P, :])
                x_tiles.append(t)
            w1_all = pool.tile([P, KC1, 512], F32, tag="w1")
            w1_r = W1.rearrange("(c p) m -> p c m", p=P)
            for k in range(KC1):
                engs[1 + k % 2].dma_start(w1_all[:, k, :], w1_r[:, k, :])
            w2_all = pool.tile([P, KC2, 256], F32, tag="w2")
            nc.gpsimd.dma_start(w2_all[:], W2.rearrange("(c p) m -> p c m", p=P))
            w3_all = pool.tile([P, KC3, FEAT_DIM], F32, tag="w3")
            nc.gpsimd.dma_start(w3_all[:], W3.rearrange("(c p) m -> p c m", p=P))
            b1_all = pool.tile([P, MC1], F32, tag="b1")
            nc.gpsimd.dma_start(b1_all[:], b1.rearrange("(mc p) -> p mc", p=P))
            b2_all = pool.tile([P, MC2], F32, tag="b2")
            nc.gpsimd.dma_start(b2_all[:], b2.rearrange("(mc p) -> p mc", p=P))
            b3_all = pool.tile([P, MC3], F32, tag="b3")
            nc.gpsimd.dma_start(b3_all[:], b3.rearrange("(mc p) -> p mc", p=P))

            h1 = _mlp_kernel(nc, tc, pool, psum, x_tiles, w1_all, b1_all,
                             KC1, MC1, True, "h1_")
            h2 = _mlp_kernel(nc, tc, pool, psum, h1, w2_all, b2_all,
                             KC2, MC2, True, "h2_")
            f_tiles = _mlp_kernel(nc, tc, pool, psum, h2, w3_all, b3_all,
                                  KC3, MC3, False, "f_")
            for m in range(MC3):
                nc.gpsimd.dma_start(fT[m * P:(m + 1) * P, :], f_tiles[m][:])

    nc.finalize()
    return nc


def _build_stage_b():
    """Task MLP, fp32r. Layer 1 keeps weights stationary (form B, small
    weights). Layers 2/3 make the ACTIVATIONS stationary and stream the big
    weight matrices as the moving operand with N=512 — this cuts LDWEIGHTS
    count 8x (LDWEIGHTS was half of all PE time in form B). The price is a
    PE-transpose of activations between layers 2 and 3, and that layer-2/3
    outputs are [batch, dim] (natural) so the final output DMAs out natural.
    """
    nc = bacc.Bacc(None, target_bir_lowering=False)
    F32R = mybir.dt.float32r
    hT = nc.declare_dram_parameter("hT", [2 * FEAT_DIM, NB], F32R, isOutput=False)
    Wt1 = nc.declare_dram_parameter("Wt1", [2 * FEAT_DIM, HID], F32R, isOutput=False)
    bt1 = nc.declare_dram_parameter("bt1", [HID], F32, isOutput=False)
    Wt2 = nc.declare_dram_parameter("Wt2", [HID, HID], F32R, isOutput=False)
    bt2 = nc.declare_dram_parameter("bt2", [HID], F32, isOutput=False)
    Wo = nc.declare_dram_parameter("Wo", [HID, OUT_DIM], F32R, isOutput=False)
    bo = nc.declare_dram_parameter("bo", [OUT_DIM], F32, isOutput=False)
    out = nc.declare_dram_parameter("out", [NB, OUT_DIM], F32, isOutput=True)
    KC1, MC1 = (2 * FEAT_DIM) // P, HID // P  # 4, 16
    KH = HID // P                             # 16
    BC = NB // P                              # 2 batch chunks
    NW2 = HID // 512                          # 4 col-chunks (L2)
    NW3 = OUT_DIM // 512                      # 2 col-chunks (L3)

    from concourse.masks import make_identity

    with tile.TileContext(nc) as tc:
        with (
            tc.tile_pool(name="sbuf", bufs=1) as pool,
            tc.tile_pool(name="stream", bufs=4) as stream,
        ):
            engs = [nc.gpsimd, nc.sync, nc.scalar]
            h_tiles = []
            for k in range(KC1):
                t = pool.tile([P, NB], F32R, tag=f"h{k}")
                nc.gpsimd.dma_start(t[:], hT[k * P:(k + 1) * P, :])
                h_tiles.append(t)
            wt1_all = pool.tile([P, KC1, HID], F32R, tag="wt1")
            wt1_r = Wt1.rearrange("(c p) m -> p c m", p=P)
            for k in range(KC1):
                nc.gpsimd.dma_start(wt1_all[:, k, :], wt1_r[:, k, :])
            bt1_all = pool.tile([P, MC1], F32, tag="bt1")
            nc.sync.dma_start(bt1_all[:], bt1.rearrange("(mc p) -> p mc", p=P))
            # biases for layers 2/3 sit on the FREE axis (natural layout), so
            # replicate them across all partitions via a broadcast DMA
            bt2_rep = pool.tile([P, HID], F32, tag="bt2")
            nc.sync.dma_start(
                bt2_rep[:],
                bt2.rearrange("(o m) -> o m", o=1).to_broadcast((P, HID)))
            bo_rep = pool.tile([P, OUT_DIM], F32, tag="bo")
            nc.sync.dma_start(
                bo_rep[:],
                bo.rearrange("(o m) -> o m", o=1).to_broadcast((P, OUT_DIM)))

            ident_f = pool.tile([P, P], F32, tag="identf")
            make_identity(nc, ident_f[:])
            ident_r = pool.tile([P, P], F32R, tag="identr")
            nc.vector.tensor_copy(ident_r[:], ident_f[:])

            # ---- layer 1 (form B): h2T[k] = relu(Wt1.T @ h + bt1), [P, NB]
            with tc.tile_pool(name="psum_a", bufs=2, space="PSUM") as psum_a:
                h2 = _mlp_kernel(nc, tc, pool, psum_a, h_tiles, wt1_all, bt1_all,
                                 KC1, MC1, True, "h2_", mm_dt=F32R)

            # ---- layer 2 (form A): h3_nat[b][n] = relu(h2T.T @ Wt2 + bt2)
            h3_nat = [[None] * NW2 for _ in range(BC)]
            with tc.tile_pool(name="psum_b", bufs=1, space="PSUM") as psum_b:
                accs = {}
                for k in range(KH):
                    w_blk = stream.tile([P, HID], F32R, tag="wt2row")
                    nc.gpsimd.dma_start(w_blk[:], Wt2[k * P:(k + 1) * P, :])
                    for b in range(BC):
                        for n in range(NW2):
                            if k == 0:
                                accs[b, n] = psum_b.tile([P, 512], F32, name=f"l2a{b}{n}",
                                                         tag=f"l2a{b}{n}")
                            nc.tensor.matmul(
                                accs[b, n][:],
                                h2[k][:, b * P:(b + 1) * P],
                                w_blk[:, n * 512:(n + 1) * 512],
                                start=(k == 0), stop=(k == KH - 1))
                for b in range(BC):
                    for n in range(NW2):
                        tmp = pool.tile([P, 512], F32, tag="l2tmp")
                        nc.vector.tensor_add(tmp[:], accs[b, n][:],
                                             bt2_rep[:, n * 512:(n + 1) * 512])
                        h3_nat[b][n] = pool.tile([P, 512], F32R, name=f"h3n{b}{n}",
                                                 tag=f"h3n{b}{n}")
                        nc.scalar.activation(h3_nat[b][n][:], tmp[:], RELU)

            # ---- transpose h3_nat -> h3T[k] [P, NB] (hid on partitions)
            h3T = [pool.tile([P, NB], F32R, name=f"h3T{k}", tag=f"h3T{k}") for k in range(KH)]
            with (
                tc.tile_pool(name="psum_t", bufs=4, space="PSUM") as psum_t,
                tc.tile_pool(name="psum_c", bufs=1, space="PSUM") as psum_c,
            ):
                for b in range(BC):
                    for kk in range(KH):
                        n, c = divmod(kk, 4)
                        ps = psum_t.tile([P, P], F32R, tag="tp")
                        nc.tensor.transpose(
                            ps[:], h3_nat[b][n][:, c * P:(c + 1) * P], ident_r[:])
                        nc.vector.tensor_copy(h3T[kk][:, b * P:(b + 1) * P], ps[:])

                # ---- layer 3 (form A): out[b][n] = h3T.T @ Wo + bo
                accs3 = {}
                for k in range(KH):
                    w_blk = stream.tile([P, OUT_DIM], F32R, tag="worow")
                    nc.gpsimd.dma_start(w_blk[:], Wo[k * P:(k + 1) * P, :])
                    for b in range(BC):
                        for n in range(NW3):
                            if k == 0:
                                accs3[b, n] = psum_c.tile([P, 512], F32, name=f"l3a{b}{n}",
                                                          tag=f"l3a{b}{n}")
                            nc.tensor.matmul(
                                accs3[b, n][:],
                                h3T[k][:, b * P:(b + 1) * P],
                                w_blk[:, n * 512:(n + 1) * 512],
                                start=(k == 0), stop=(k == KH - 1))
                for b in range(BC):
                    for n in range(NW3):
                        o_t = pool.tile([P, 512], F32, tag="l3o")
                        nc.vector.tensor_add(o_t[:], accs3[b, n][:],
                                             bo_rep[:, n * 512:(n + 1) * 512])
                        nc.gpsimd.dma_start(
                            out[b * P:(b + 1) * P, n * 512:(n + 1) * 512], o_t[:])

    nc.finalize()
    return nc


_CACHE = {}


def _stage(name, builder):
    if name not in _CACHE:
        _CACHE[name] = builder()
    return _CACHE[name]


def kernel(x, W1, b1, W2, b2, W3, b3, memory, Wt1, bt1, Wt2, bt2, Wo, bo,
           _collect=None, _trace=False):
    x = np.asarray(x, dtype=np.float32)
    W1 = np.asarray(W1, dtype=np.float32); b1 = np.asarray(b1, dtype=np.float32)
    W2 = np.asarray(W2, dtype=np.float32); b2 = np.asarray(b2, dtype=np.float32)
    W3 = np.asarray(W3, dtype=np.float32); b3 = np.asarray(b3, dtype=np.float32)
    memory = np.asarray(memory, dtype=np.float32)
    Wt1 = np.asarray(Wt1, dtype=np.float32); bt1 = np.asarray(bt1, dtype=np.float32)
    Wt2 = np.asarray(Wt2, dtype=np.float32); bt2 = np.asarray(bt2, dtype=np.float32)
    Wo = np.asarray(Wo, dtype=np.float32); bo = np.asarray(bo, dtype=np.float32)

    # ---- stage A: feature MLP on 8 cores (batch-sharded) ----
    xT = np.ascontiguousarray(x.T)  # [IN_DIM, B]
    nc_a = _stage("a", _build_stage_a)
    in_maps = [
        {"xT": np.ascontiguousarray(xT[:, c * NB:(c + 1) * NB]),
         "W1": W1, "b1": b1, "W2": W2, "b2": b2, "W3": W3, "b3": b3}
        for c in range(NCORES)
    ]
    res_a = run_bass_kernel_spmd(nc_a, in_maps, list(range(NCORES)), trace=_trace)
    fT = np.concatenate([res_a.results[c]["fT"] for c in range(NCORES)], axis=1)
    f = np.ascontiguousarray(fT.T)  # [B, FEAT_DIM]

    # ---- host glue: scores + top-k, mirroring the reference's ops ----
    import jax
    import jax.numpy as jnp
    fd = jnp.asarray(f)
    md = jnp.asarray(memory)
    bsz = B
    f_sum = jnp.sum(fd, axis=0)
    f_sq = jnp.sum(fd * fd)
    m_sq = jnp.sum(md * md, axis=1)
    scores = -(bsz * m_sq - 2.0 * (md @ f_sum) + f_sq)
    _, idx = jax.lax.top_k(scores, B)
    idx = np.asarray(idx)
    mem_out = memory[idx]  # [B, FEAT_DIM]

    # ---- stage B: task MLP on 8 cores (batch-sharded) ----
    hT = np.ascontiguousarray(np.concatenate([f, mem_out], axis=1).T)  # [512, B]
    nc_b = _stage("b", _build_stage_b)
    in_maps = [
        {"hT": np.ascontiguousarray(hT[:, c * NB:(c + 1) * NB]),
         "Wt1": Wt1, "bt1": bt1, "Wt2": Wt2, "bt2": bt2, "Wo": Wo, "bo": bo}
        for c in range(NCORES)
    ]
    res_b = run_bass_kernel_spmd(nc_b, in_maps, list(range(NCORES)), trace=_trace)
    out = np.concatenate([res_b.results[c]["out"] for c in range(NCORES)], axis=0)

    if _collect is not None:
        _collect["f"] = f
        _collect["idx"] = idx
        _collect["res_a"] = res_a
        _collect["res_b"] = res_b
    return out


# revision 21
# speedup vs baseline: 1.0360x; 1.0026x over previous
"""nn_ContinualLearningNetwork — Trainium2 Bass SPMD kernel (8 NeuronCores).

Structure:
  Stage A (bass, 8 cores, batch-sharded): feature MLP 2048->512->256->256, fp32,
    transposed-activation layout (activations [dim, batch] on chip).
  Host glue (jnp, same backend/ops as the reference): memory scores + top-k.
    The top-k ordering of 100k scores has near-ties at fp32 resolution; the
    B*m_sq term amplifies m_sq rounding 2048x, so m_sq/scores must be computed
    with the reference's own op sequence to reproduce its ordering.
  Stage B (bass, 8 cores, batch-sharded): task MLP 512->2048->2048->1024, fp32r.
"""

import numpy as np

import concourse.bacc as bacc
import concourse.mybir as mybir
import concourse.tile as tile
from concourse.bass_utils import run_bass_kernel_spmd

P = 128
NCORES = 8
B = 2048
IN_DIM = 2048
FEAT_DIM = 256
HID = 2048
OUT_DIM = 1024
NB = B // NCORES  # 256 batch rows per core

F32 = mybir.dt.float32
RELU = mybir.ActivationFunctionType.Relu
COPY = mybir.ActivationFunctionType.Identity


def _mlp_kernel(nc, tc, pool, psum, rhs_tiles, w_all, b_all, k_chunks, m_chunks,
                relu, out_prefix, mm_dt=None):
    """Emit one dense layer: out[m][128, NB] = act(W.T @ rhs + b).

    rhs_tiles: list of k_chunks SBUF tiles [P, NB] (transposed activations).
    w_all: SBUF tile [P, k_chunks, M] (natural [K, M] weight layout).
    b_all: SBUF tile [P, m_chunks] (bias, partition-major per m-chunk).
    mm_dt: bitcast matmul operands to this dtype (float32r runs 4x faster
      than float32 at ~1.5e-4 precision; fine for the task MLP, not for the
      feature MLP whose output feeds the top-k score ordering).
    Returns list of m_chunks SBUF tiles [P, NB].
    """
    outs = []
    for m in range(m_chunks):
        acc = psum.tile([P, NB], F32, tag="acc")
        for k in range(k_chunks):
            nc.tensor.matmul(
                acc[:],
                w_all[:, k, m * P:(m + 1) * P],
                rhs_tiles[k][:],
                start=(k == 0),
                stop=(k == k_chunks - 1),
            )
        o_t = pool.tile([P, NB], mm_dt or F32, tag=f"{out_prefix}{m}")
        nc.scalar.activation(o_t[:], acc[:], RELU if relu else COPY,
                             bias=b_all[:, m:m + 1])
        outs.append(o_t)
    return outs


def _build_stage_a():
    nc = bacc.Bacc(None, target_bir_lowering=False)
    xT = nc.declare_dram_parameter("xT", [IN_DIM, NB], F32, isOutput=False)
    W1 = nc.declare_dram_parameter("W1", [IN_DIM, 512], F32, isOutput=False)
    b1 = nc.declare_dram_parameter("b1", [512], F32, isOutput=False)
    W2 = nc.declare_dram_parameter("W2", [512, 256], F32, isOutput=False)
    b2 = nc.declare_dram_parameter("b2", [256], F32, isOutput=False)
    W3 = nc.declare_dram_parameter("W3", [256, FEAT_DIM], F32, isOutput=False)
    b3 = nc.declare_dram_parameter("b3", [FEAT_DIM], F32, isOutput=False)
    fT = nc.declare_dram_parameter("fT", [FEAT_DIM, NB], F32, isOutput=True)

    KC1, MC1 = IN_DIM // P, 512 // P    # 16, 4
    KC2, MC2 = 512 // P, 256 // P       # 4, 2
    KC3, MC3 = 256 // P, FEAT_DIM // P  # 2, 2

    with tile.TileContext(nc) as tc:
        with (
            tc.tile_pool(name="sbuf", bufs=1) as pool,
            tc.tile_pool(name="psum", bufs=4, space="PSUM") as psum,
        ):
            engs = [nc.gpsimd, nc.sync, nc.scalar]
            x_tiles = []
            for k in range(KC1):
                t = pool.tile([P, NB], F32, tag=f"x{k}")
                engs[k % 2].dma_start(t[:], xT[k * P:(k + 1) * P, :])
                x_tiles.append(t)
            w1_all = pool.tile([P, KC1, 512], F32, tag="w1")
            w1_r = W1.rearrange("(c p) m -> p c m", p=P)
            for k in range(KC1):
                engs[1 + k % 2].dma_start(w1_all[:, k, :], w1_r[:, k, :])
            w2_all = pool.tile([P, KC2, 256], F32, tag="w2")
            nc.gpsimd.dma_start(w2_all[:], W2.rearrange("(c p) m -> p c m", p=P))
            w3_all = pool.tile([P, KC3, FEAT_DIM], F32, tag="w3")
            nc.gpsimd.dma_start(w3_all[:], W3.rearrange("(c p) m -> p c m", p=P))
            b1_all = pool.tile([P, MC1], F32, tag="b1")
            nc.gpsimd.dma_start(b1_all[:], b1.rearrange("(mc p) -> p mc", p=P))
            b2_all = pool.tile([P, MC2], F32, tag="b2")
            nc.gpsimd.dma_start(b2_all[:], b2.rearrange("(mc p) -> p mc", p=P))
            b3_all = pool.tile([P, MC3], F32, tag="b3")
            nc.gpsimd.dma_start(b3_all[:], b3.rearrange("(mc p) -> p mc", p=P))

            h1 = _mlp_kernel(nc, tc, pool, psum, x_tiles, w1_all, b1_all,
                             KC1, MC1, True, "h1_")
            h2 = _mlp_kernel(nc, tc, pool, psum, h1, w2_all, b2_all,
                             KC2, MC2, True, "h2_")
            f_tiles = _mlp_kernel(nc, tc, pool, psum, h2, w3_all, b3_all,
                                  KC3, MC3, False, "f_")
            for m in range(MC3):
                nc.gpsimd.dma_start(fT[m * P:(m + 1) * P, :], f_tiles[m][:])

    nc.finalize()
    return nc


def _build_stage_b():
    """Task MLP, fp32r. Layer 1 keeps weights stationary (form B, small
    weights). Layers 2/3 make the ACTIVATIONS stationary and stream the big
    weight matrices as the moving operand with N=512 — this cuts LDWEIGHTS
    count 8x (LDWEIGHTS was half of all PE time in form B). The price is a
    PE-transpose of activations between layers 2 and 3, and that layer-2/3
    outputs are [batch, dim] (natural) so the final output DMAs out natural.
    """
    nc = bacc.Bacc(None, target_bir_lowering=False)
    F32R = mybir.dt.float32r
    hT = nc.declare_dram_parameter("hT", [2 * FEAT_DIM, NB], F32R, isOutput=False)
    Wt1 = nc.declare_dram_parameter("Wt1", [2 * FEAT_DIM, HID], F32R, isOutput=False)
    bt1 = nc.declare_dram_parameter("bt1", [HID], F32, isOutput=False)
    Wt2 = nc.declare_dram_parameter("Wt2", [HID, HID], F32R, isOutput=False)
    bt2 = nc.declare_dram_parameter("bt2", [HID], F32, isOutput=False)
    Wo = nc.declare_dram_parameter("Wo", [HID, OUT_DIM], F32R, isOutput=False)
    bo = nc.declare_dram_parameter("bo", [OUT_DIM], F32, isOutput=False)
    out = nc.declare_dram_parameter("out", [NB, OUT_DIM], F32, isOutput=True)
    KC1, MC1 = (2 * FEAT_DIM) // P, HID // P  # 4, 16
    KH = HID // P                             # 16
    BC = NB // P                              # 2 batch chunks
    NW2 = HID // 512                          # 4 col-chunks (L2)
    NW3 = OUT_DIM // 512                      # 2 col-chunks (L3)

    from concourse.masks import make_identity

    with tile.TileContext(nc) as tc:
        with (
            tc.tile_pool(name="sbuf", bufs=1) as pool,
            tc.tile_pool(name="stream", bufs=4) as stream,
        ):
            engs = [nc.gpsimd, nc.sync, nc.scalar]
            h_tiles = []
            for k in range(KC1):
                t = pool.tile([P, NB], F32R, tag=f"h{k}")
                nc.gpsimd.dma_start(t[:], hT[k * P:(k + 1) * P, :])
                h_tiles.append(t)
            wt1_all = pool.tile([P, KC1, HID], F32R, tag="wt1")
            wt1_r = Wt1.rearrange("(c p) m -> p c m", p=P)
            for k in range(KC1):
                nc.gpsimd.dma_start(wt1_all[:, k, :], wt1_r[:, k, :])
            bt1_all = pool.tile([P, MC1], F32, tag="bt1")
            nc.gpsimd.dma_start(bt1_all[:], bt1.rearrange("(mc p) -> p mc", p=P))
            # biases for layers 2/3 sit on the FREE axis (natural layout), so
            # replicate them across all partitions via a broadcast DMA
            bt2_rep = pool.tile([P, HID], F32, tag="bt2")
            nc.gpsimd.dma_start(
                bt2_rep[:],
                bt2.rearrange("(o m) -> o m", o=1).to_broadcast((P, HID)))
            bo_rep = pool.tile([P, OUT_DIM], F32, tag="bo")
            nc.gpsimd.dma_start(
                bo_rep[:],
                bo.rearrange("(o m) -> o m", o=1).to_broadcast((P, OUT_DIM)))

            ident_f = pool.tile([P, P], F32, tag="identf")
            make_identity(nc, ident_f[:])
            ident_r = pool.tile([P, P], F32R, tag="identr")
            nc.vector.tensor_copy(ident_r[:], ident_f[:])

            # ---- layer 1 (form B): h2T[k] = relu(Wt1.T @ h + bt1), [P, NB]
            with tc.tile_pool(name="psum_a", bufs=2, space="PSUM") as psum_a:
                h2 = _mlp_kernel(nc, tc, pool, psum_a, h_tiles, wt1_all, bt1_all,
                                 KC1, MC1, True, "h2_", mm_dt=F32R)

            # ---- layer 2 (form A): h3_nat[b][n] = relu(h2T.T @ Wt2 + bt2)
            h3_nat = [[None] * NW2 for _ in range(BC)]
            with tc.tile_pool(name="psum_b", bufs=1, space="PSUM") as psum_b:
                accs = {}
                for k in range(KH):
                    w_blk = stream.tile([P, HID], F32R, tag="wt2row")
                    nc.gpsimd.dma_start(w_blk[:], Wt2[k * P:(k + 1) * P, :])
                    for b in range(BC):
                        for n in range(NW2):
                            if k == 0:
                                accs[b, n] = psum_b.tile([P, 512], F32, name=f"l2a{b}{n}",
                                                         tag=f"l2a{b}{n}")
                            nc.tensor.matmul(
                                accs[b, n][:],
                                h2[k][:, b * P:(b + 1) * P],
                                w_blk[:, n * 512:(n + 1) * 512],
                                start=(k == 0), stop=(k == KH - 1))
                for b in range(BC):
                    for n in range(NW2):
                        tmp = pool.tile([P, 512], F32, tag="l2tmp")
                        nc.vector.tensor_add(tmp[:], accs[b, n][:],
                                             bt2_rep[:, n * 512:(n + 1) * 512])
                        h3_nat[b][n] = pool.tile([P, 512], F32R, name=f"h3n{b}{n}",
                                                 tag=f"h3n{b}{n}")
                        nc.scalar.activation(h3_nat[b][n][:], tmp[:], RELU)

            # ---- transpose h3_nat -> h3T[k] [P, NB] (hid on partitions)
            h3T = [pool.tile([P, NB], F32R, name=f"h3T{k}", tag=f"h3T{k}") for k in range(KH)]
            with (
                tc.tile_pool(name="psum_t", bufs=4, space="PSUM") as psum_t,
                tc.tile_pool(name="psum_c", bufs=1, space="PSUM") as psum_c,
            ):
                for b in range(BC):
                    for kk in range(KH):
                        n, c = divmod(kk, 4)
                        ps = psum_t.tile([P, P], F32R, tag="tp")
                        nc.tensor.transpose(
                            ps[:], h3_nat[b][n][:, c * P:(c + 1) * P], ident_r[:])
                        nc.vector.tensor_copy(h3T[kk][:, b * P:(b + 1) * P], ps[:])

                # ---- layer 3 (form A): out[b][n] = h3T.T @ Wo + bo
                accs3 = {}
                for k in range(KH):
                    w_blk = stream.tile([P, OUT_DIM], F32R, tag="worow")
                    nc.gpsimd.dma_start(w_blk[:], Wo[k * P:(k + 1) * P, :])
                    for b in range(BC):
                        for n in range(NW3):
                            if k == 0:
                                accs3[b, n] = psum_c.tile([P, 512], F32, name=f"l3a{b}{n}",
                                                          tag=f"l3a{b}{n}")
                            nc.tensor.matmul(
                                accs3[b, n][:],
                                h3T[k][:, b * P:(b + 1) * P],
                                w_blk[:, n * 512:(n + 1) * 512],
                                start=(k == 0), stop=(k == KH - 1))
                for b in range(BC):
                    for n in range(NW3):
                        o_t = pool.tile([P, 512], F32, tag="l3o")
                        nc.vector.tensor_add(o_t[:], accs3[b, n][:],
                                             bo_rep[:, n * 512:(n + 1) * 512])
                        nc.gpsimd.dma_start(
                            out[b * P:(b + 1) * P, n * 512:(n + 1) * 512], o_t[:])

    nc.finalize()
    return nc


_CACHE = {}


def _stage(name, builder):
    if name not in _CACHE:
        _CACHE[name] = builder()
    return _CACHE[name]


def kernel(x, W1, b1, W2, b2, W3, b3, memory, Wt1, bt1, Wt2, bt2, Wo, bo,
           _collect=None, _trace=False):
    x = np.asarray(x, dtype=np.float32)
    W1 = np.asarray(W1, dtype=np.float32); b1 = np.asarray(b1, dtype=np.float32)
    W2 = np.asarray(W2, dtype=np.float32); b2 = np.asarray(b2, dtype=np.float32)
    W3 = np.asarray(W3, dtype=np.float32); b3 = np.asarray(b3, dtype=np.float32)
    memory = np.asarray(memory, dtype=np.float32)
    Wt1 = np.asarray(Wt1, dtype=np.float32); bt1 = np.asarray(bt1, dtype=np.float32)
    Wt2 = np.asarray(Wt2, dtype=np.float32); bt2 = np.asarray(bt2, dtype=np.float32)
    Wo = np.asarray(Wo, dtype=np.float32); bo = np.asarray(bo, dtype=np.float32)

    # ---- stage A: feature MLP on 8 cores (batch-sharded) ----
    xT = np.ascontiguousarray(x.T)  # [IN_DIM, B]
    nc_a = _stage("a", _build_stage_a)
    in_maps = [
        {"xT": np.ascontiguousarray(xT[:, c * NB:(c + 1) * NB]),
         "W1": W1, "b1": b1, "W2": W2, "b2": b2, "W3": W3, "b3": b3}
        for c in range(NCORES)
    ]
    res_a = run_bass_kernel_spmd(nc_a, in_maps, list(range(NCORES)), trace=_trace)
    fT = np.concatenate([res_a.results[c]["fT"] for c in range(NCORES)], axis=1)
    f = np.ascontiguousarray(fT.T)  # [B, FEAT_DIM]

    # ---- host glue: scores + top-k, mirroring the reference's ops ----
    import jax
    import jax.numpy as jnp
    fd = jnp.asarray(f)
    md = jnp.asarray(memory)
    bsz = B
    f_sum = jnp.sum(fd, axis=0)
    f_sq = jnp.sum(fd * fd)
    m_sq = jnp.sum(md * md, axis=1)
    scores = -(bsz * m_sq - 2.0 * (md @ f_sum) + f_sq)
    _, idx = jax.lax.top_k(scores, B)
    idx = np.asarray(idx)
    mem_out = memory[idx]  # [B, FEAT_DIM]

    # ---- stage B: task MLP on 8 cores (batch-sharded) ----
    hT = np.ascontiguousarray(np.concatenate([f, mem_out], axis=1).T)  # [512, B]
    nc_b = _stage("b", _build_stage_b)
    in_maps = [
        {"hT": np.ascontiguousarray(hT[:, c * NB:(c + 1) * NB]),
         "Wt1": Wt1, "bt1": bt1, "Wt2": Wt2, "bt2": bt2, "Wo": Wo, "bo": bo}
        for c in range(NCORES)
    ]
    res_b = run_bass_kernel_spmd(nc_b, in_maps, list(range(NCORES)), trace=_trace)
    out = np.concatenate([res_b.results[c]["out"] for c in range(NCORES)], axis=0)

    if _collect is not None:
        _collect["f"] = f
        _collect["idx"] = idx
        _collect["res_a"] = res_a
        _collect["res_b"] = res_b
    return out


# revision 22
# speedup vs baseline: 1.1621x; 1.1216x over previous
"""nn_ContinualLearningNetwork — Trainium2 Bass SPMD kernel (8 NeuronCores).

Structure:
  Stage A (bass, 8 cores, batch-sharded): feature MLP 2048->512->256->256, fp32,
    transposed-activation layout (activations [dim, batch] on chip).
  Host glue (jnp, same backend/ops as the reference): memory scores + top-k.
    The top-k ordering of 100k scores has near-ties at fp32 resolution; the
    B*m_sq term amplifies m_sq rounding 2048x, so m_sq/scores must be computed
    with the reference's own op sequence to reproduce its ordering.
  Stage B (bass, 8 cores, batch-sharded): task MLP 512->2048->2048->1024, fp32r.
"""

import numpy as np

import concourse.bacc as bacc
import concourse.mybir as mybir
import concourse.tile as tile
from concourse.bass_utils import run_bass_kernel_spmd

P = 128
NCORES = 8
B = 2048
IN_DIM = 2048
FEAT_DIM = 256
HID = 2048
OUT_DIM = 1024
NB = B // NCORES  # 256 batch rows per core

F32 = mybir.dt.float32
RELU = mybir.ActivationFunctionType.Relu
COPY = mybir.ActivationFunctionType.Identity


def _mlp_kernel(nc, tc, pool, psum, rhs_tiles, w_all, b_all, k_chunks, m_chunks,
                relu, out_prefix, mm_dt=None):
    """Emit one dense layer: out[m][128, NB] = act(W.T @ rhs + b).

    rhs_tiles: list of k_chunks SBUF tiles [P, NB] (transposed activations).
    w_all: SBUF tile [P, k_chunks, M] (natural [K, M] weight layout).
    b_all: SBUF tile [P, m_chunks] (bias, partition-major per m-chunk).
    mm_dt: bitcast matmul operands to this dtype (float32r runs 4x faster
      than float32 at ~1.5e-4 precision; fine for the task MLP, not for the
      feature MLP whose output feeds the top-k score ordering).
    Returns list of m_chunks SBUF tiles [P, NB].
    """
    outs = []
    for m in range(m_chunks):
        acc = psum.tile([P, NB], F32, tag="acc")
        for k in range(k_chunks):
            nc.tensor.matmul(
                acc[:],
                w_all[:, k, m * P:(m + 1) * P],
                rhs_tiles[k][:],
                start=(k == 0),
                stop=(k == k_chunks - 1),
            )
        o_t = pool.tile([P, NB], mm_dt or F32, tag=f"{out_prefix}{m}")
        nc.scalar.activation(o_t[:], acc[:], RELU if relu else COPY,
                             bias=b_all[:, m:m + 1])
        outs.append(o_t)
    return outs


def _build_stage_a():
    nc = bacc.Bacc(None, target_bir_lowering=False)
    xT = nc.declare_dram_parameter("xT", [IN_DIM, NB], F32, isOutput=False)
    W1 = nc.declare_dram_parameter("W1", [IN_DIM, 512], F32, isOutput=False)
    b1 = nc.declare_dram_parameter("b1", [512], F32, isOutput=False)
    W2 = nc.declare_dram_parameter("W2", [512, 256], F32, isOutput=False)
    b2 = nc.declare_dram_parameter("b2", [256], F32, isOutput=False)
    W3 = nc.declare_dram_parameter("W3", [256, FEAT_DIM], F32, isOutput=False)
    b3 = nc.declare_dram_parameter("b3", [FEAT_DIM], F32, isOutput=False)
    fT = nc.declare_dram_parameter("fT", [FEAT_DIM, NB], F32, isOutput=True)

    KC1, MC1 = IN_DIM // P, 512 // P    # 16, 4
    KC2, MC2 = 512 // P, 256 // P       # 4, 2
    KC3, MC3 = 256 // P, FEAT_DIM // P  # 2, 2

    with tile.TileContext(nc) as tc:
        with (
            tc.tile_pool(name="sbuf", bufs=1) as pool,
            tc.tile_pool(name="psum", bufs=4, space="PSUM") as psum,
        ):
            engs = [nc.gpsimd, nc.sync, nc.scalar]
            x_tiles = []
            for k in range(KC1):
                t = pool.tile([P, NB], F32, tag=f"x{k}")
                engs[k % 2].dma_start(t[:], xT[k * P:(k + 1) * P, :])
                x_tiles.append(t)
            w1_all = pool.tile([P, KC1, 512], F32, tag="w1")
            w1_r = W1.rearrange("(c p) m -> p c m", p=P)
            for k in range(KC1):
                engs[1 + k % 2].dma_start(w1_all[:, k, :], w1_r[:, k, :])
            w2_all = pool.tile([P, KC2, 256], F32, tag="w2")
            nc.gpsimd.dma_start(w2_all[:], W2.rearrange("(c p) m -> p c m", p=P))
            w3_all = pool.tile([P, KC3, FEAT_DIM], F32, tag="w3")
            nc.gpsimd.dma_start(w3_all[:], W3.rearrange("(c p) m -> p c m", p=P))
            b1_all = pool.tile([P, MC1], F32, tag="b1")
            nc.gpsimd.dma_start(b1_all[:], b1.rearrange("(mc p) -> p mc", p=P))
            b2_all = pool.tile([P, MC2], F32, tag="b2")
            nc.gpsimd.dma_start(b2_all[:], b2.rearrange("(mc p) -> p mc", p=P))
            b3_all = pool.tile([P, MC3], F32, tag="b3")
            nc.gpsimd.dma_start(b3_all[:], b3.rearrange("(mc p) -> p mc", p=P))

            h1 = _mlp_kernel(nc, tc, pool, psum, x_tiles, w1_all, b1_all,
                             KC1, MC1, True, "h1_")
            h2 = _mlp_kernel(nc, tc, pool, psum, h1, w2_all, b2_all,
                             KC2, MC2, True, "h2_")
            f_tiles = _mlp_kernel(nc, tc, pool, psum, h2, w3_all, b3_all,
                                  KC3, MC3, False, "f_")
            for m in range(MC3):
                nc.gpsimd.dma_start(fT[m * P:(m + 1) * P, :], f_tiles[m][:])

    nc.finalize()
    return nc


def _build_stage_b():
    """Task MLP, fp32r. Layer 1 keeps weights stationary (form B, small
    weights). Layers 2/3 make the ACTIVATIONS stationary and stream the big
    weight matrices as the moving operand with N=512 — this cuts LDWEIGHTS
    count 8x (LDWEIGHTS was half of all PE time in form B). The price is a
    PE-transpose of activations between layers 2 and 3, and that layer-2/3
    outputs are [batch, dim] (natural) so the final output DMAs out natural.
    """
    nc = bacc.Bacc(None, target_bir_lowering=False)
    F32R = mybir.dt.float32r
    hT = nc.declare_dram_parameter("hT", [2 * FEAT_DIM, NB], F32R, isOutput=False)
    Wt1 = nc.declare_dram_parameter("Wt1", [2 * FEAT_DIM, HID], F32R, isOutput=False)
    bt1 = nc.declare_dram_parameter("bt1", [HID], F32, isOutput=False)
    Wt2 = nc.declare_dram_parameter("Wt2", [HID, HID], F32R, isOutput=False)
    bt2 = nc.declare_dram_parameter("bt2", [HID], F32, isOutput=False)
    Wo = nc.declare_dram_parameter("Wo", [HID, OUT_DIM], F32R, isOutput=False)
    bo = nc.declare_dram_parameter("bo", [OUT_DIM], F32, isOutput=False)
    out = nc.declare_dram_parameter("out", [NB, OUT_DIM], F32, isOutput=True)
    KC1, MC1 = (2 * FEAT_DIM) // P, HID // P  # 4, 16
    KH = HID // P                             # 16
    BC = NB // P                              # 2 batch chunks
    NW2 = HID // 512                          # 4 col-chunks (L2)
    NW3 = OUT_DIM // 512                      # 2 col-chunks (L3)

    from concourse.masks import make_identity

    with tile.TileContext(nc) as tc:
        with (
            tc.tile_pool(name="sbuf", bufs=1) as pool,
            tc.tile_pool(name="stream", bufs=6) as stream,
        ):
            engs = [nc.gpsimd, nc.sync, nc.scalar]
            h_tiles = []
            for k in range(KC1):
                t = pool.tile([P, NB], F32R, tag=f"h{k}")
                nc.gpsimd.dma_start(t[:], hT[k * P:(k + 1) * P, :])
                h_tiles.append(t)
            wt1_all = pool.tile([P, KC1, HID], F32R, tag="wt1")
            wt1_r = Wt1.rearrange("(c p) m -> p c m", p=P)
            for k in range(KC1):
                nc.gpsimd.dma_start(wt1_all[:, k, :], wt1_r[:, k, :])
            bt1_all = pool.tile([P, MC1], F32, tag="bt1")
            nc.gpsimd.dma_start(bt1_all[:], bt1.rearrange("(mc p) -> p mc", p=P))
            # biases for layers 2/3 sit on the FREE axis (natural layout), so
            # replicate them across all partitions via a broadcast DMA
            bt2_rep = pool.tile([P, HID], F32, tag="bt2")
            nc.gpsimd.dma_start(
                bt2_rep[:],
                bt2.rearrange("(o m) -> o m", o=1).to_broadcast((P, HID)))
            bo_rep = pool.tile([P, OUT_DIM], F32, tag="bo")
            nc.gpsimd.dma_start(
                bo_rep[:],
                bo.rearrange("(o m) -> o m", o=1).to_broadcast((P, OUT_DIM)))

            ident_f = pool.tile([P, P], F32, tag="identf")
            make_identity(nc, ident_f[:])
            ident_r = pool.tile([P, P], F32R, tag="identr")
            nc.vector.tensor_copy(ident_r[:], ident_f[:])

            # ---- layer 1 (form B): h2T[k] = relu(Wt1.T @ h + bt1), [P, NB]
            with tc.tile_pool(name="psum_a", bufs=2, space="PSUM") as psum_a:
                h2 = _mlp_kernel(nc, tc, pool, psum_a, h_tiles, wt1_all, bt1_all,
                                 KC1, MC1, True, "h2_", mm_dt=F32R)

            # ---- layer 2 (form A): h3_nat[b][n] = relu(h2T.T @ Wt2 + bt2)
            h3_nat = [[None] * NW2 for _ in range(BC)]
            with tc.tile_pool(name="psum_b", bufs=1, space="PSUM") as psum_b:
                accs = {}
                for k in range(KH):
                    w_blk = stream.tile([P, HID], F32R, tag="wt2row")
                    nc.gpsimd.dma_start(w_blk[:], Wt2[k * P:(k + 1) * P, :])
                    for b in range(BC):
                        for n in range(NW2):
                            if k == 0:
                                accs[b, n] = psum_b.tile([P, 512], F32, name=f"l2a{b}{n}",
                                                         tag=f"l2a{b}{n}")
                            nc.tensor.matmul(
                                accs[b, n][:],
                                h2[k][:, b * P:(b + 1) * P],
                                w_blk[:, n * 512:(n + 1) * 512],
                                start=(k == 0), stop=(k == KH - 1))
                for b in range(BC):
                    for n in range(NW2):
                        tmp = pool.tile([P, 512], F32, tag="l2tmp")
                        nc.vector.tensor_add(tmp[:], accs[b, n][:],
                                             bt2_rep[:, n * 512:(n + 1) * 512])
                        h3_nat[b][n] = pool.tile([P, 512], F32R, name=f"h3n{b}{n}",
                                                 tag=f"h3n{b}{n}")
                        nc.scalar.activation(h3_nat[b][n][:], tmp[:], RELU)

            # ---- transpose h3_nat -> h3T[k] [P, NB] (hid on partitions)
            h3T = [pool.tile([P, NB], F32R, name=f"h3T{k}", tag=f"h3T{k}") for k in range(KH)]
            with (
                tc.tile_pool(name="psum_t", bufs=4, space="PSUM") as psum_t,
                tc.tile_pool(name="psum_c", bufs=1, space="PSUM") as psum_c,
            ):
                for b in range(BC):
                    for kk in range(KH):
                        n, c = divmod(kk, 4)
                        ps = psum_t.tile([P, P], F32R, tag="tp")
                        nc.tensor.transpose(
                            ps[:], h3_nat[b][n][:, c * P:(c + 1) * P], ident_r[:])
                        nc.vector.tensor_copy(h3T[kk][:, b * P:(b + 1) * P], ps[:])

                # ---- layer 3 (form A): out[b][n] = h3T.T @ Wo + bo
                accs3 = {}
                for k in range(KH):
                    w_blk = stream.tile([P, OUT_DIM], F32R, tag="worow")
                    nc.gpsimd.dma_start(w_blk[:], Wo[k * P:(k + 1) * P, :])
                    for b in range(BC):
                        for n in range(NW3):
                            if k == 0:
                                accs3[b, n] = psum_c.tile([P, 512], F32, name=f"l3a{b}{n}",
                                                          tag=f"l3a{b}{n}")
                            nc.tensor.matmul(
                                accs3[b, n][:],
                                h3T[k][:, b * P:(b + 1) * P],
                                w_blk[:, n * 512:(n + 1) * 512],
                                start=(k == 0), stop=(k == KH - 1))
                for b in range(BC):
                    for n in range(NW3):
                        o_t = pool.tile([P, 512], F32, tag="l3o")
                        nc.vector.tensor_add(o_t[:], accs3[b, n][:],
                                             bo_rep[:, n * 512:(n + 1) * 512])
                        nc.gpsimd.dma_start(
                            out[b * P:(b + 1) * P, n * 512:(n + 1) * 512], o_t[:])

    nc.finalize()
    return nc


_CACHE = {}


def _stage(name, builder):
    if name not in _CACHE:
        _CACHE[name] = builder()
    return _CACHE[name]


def kernel(x, W1, b1, W2, b2, W3, b3, memory, Wt1, bt1, Wt2, bt2, Wo, bo,
           _collect=None, _trace=False):
    x = np.asarray(x, dtype=np.float32)
    W1 = np.asarray(W1, dtype=np.float32); b1 = np.asarray(b1, dtype=np.float32)
    W2 = np.asarray(W2, dtype=np.float32); b2 = np.asarray(b2, dtype=np.float32)
    W3 = np.asarray(W3, dtype=np.float32); b3 = np.asarray(b3, dtype=np.float32)
    memory = np.asarray(memory, dtype=np.float32)
    Wt1 = np.asarray(Wt1, dtype=np.float32); bt1 = np.asarray(bt1, dtype=np.float32)
    Wt2 = np.asarray(Wt2, dtype=np.float32); bt2 = np.asarray(bt2, dtype=np.float32)
    Wo = np.asarray(Wo, dtype=np.float32); bo = np.asarray(bo, dtype=np.float32)

    # ---- stage A: feature MLP on 8 cores (batch-sharded) ----
    xT = np.ascontiguousarray(x.T)  # [IN_DIM, B]
    nc_a = _stage("a", _build_stage_a)
    in_maps = [
        {"xT": np.ascontiguousarray(xT[:, c * NB:(c + 1) * NB]),
         "W1": W1, "b1": b1, "W2": W2, "b2": b2, "W3": W3, "b3": b3}
        for c in range(NCORES)
    ]
    res_a = run_bass_kernel_spmd(nc_a, in_maps, list(range(NCORES)), trace=_trace)
    fT = np.concatenate([res_a.results[c]["fT"] for c in range(NCORES)], axis=1)
    f = np.ascontiguousarray(fT.T)  # [B, FEAT_DIM]

    # ---- host glue: scores + top-k, mirroring the reference's ops ----
    import jax
    import jax.numpy as jnp
    fd = jnp.asarray(f)
    md = jnp.asarray(memory)
    bsz = B
    f_sum = jnp.sum(fd, axis=0)
    f_sq = jnp.sum(fd * fd)
    m_sq = jnp.sum(md * md, axis=1)
    scores = -(bsz * m_sq - 2.0 * (md @ f_sum) + f_sq)
    _, idx = jax.lax.top_k(scores, B)
    idx = np.asarray(idx)
    mem_out = memory[idx]  # [B, FEAT_DIM]

    # ---- stage B: task MLP on 8 cores (batch-sharded) ----
    hT = np.ascontiguousarray(np.concatenate([f, mem_out], axis=1).T)  # [512, B]
    nc_b = _stage("b", _build_stage_b)
    in_maps = [
        {"hT": np.ascontiguousarray(hT[:, c * NB:(c + 1) * NB]),
         "Wt1": Wt1, "bt1": bt1, "Wt2": Wt2, "bt2": bt2, "Wo": Wo, "bo": bo}
        for c in range(NCORES)
    ]
    res_b = run_bass_kernel_spmd(nc_b, in_maps, list(range(NCORES)), trace=_trace)
    out = np.concatenate([res_b.results[c]["out"] for c in range(NCORES)], axis=0)

    if _collect is not None:
        _collect["f"] = f
        _collect["idx"] = idx
        _collect["res_a"] = res_a
        _collect["res_b"] = res_b
    return out
